# revision 45
# baseline (speedup 1.0000x reference)
"""HSTU block kernel v6 for 8 TRN2 NeuronCores (nn_HSTU_66279935312625).

Sharding: 2 cores per batch (B=4). The 2048 rows are split at 256-row
granularity with the causally-balanced ABBA pattern: core g=0 owns
256-blocks {0,3,4,7}, g=1 owns {1,2,5,6}. Sorted by causal extent,
both cores' four 256-row chunks need at most (4,8,12,16) 128-row
k-blocks, so ONE program (fixed loop bounds) fits both cores; all
per-core variation lives in host data (row gather + masks).

v6 vs v4/v5:
- attention runs per (chunk, head-pair) with per-chunk k-extent
  (4,8,12,16) instead of (8,16) for 512-row units: 17% fewer sigmoid
  elements (ACT is the stream bottleneck at 1 elem/cycle/lane), 4x
  fewer DVE mask multiplies (only the last 4 k-blocks of each chunk
  straddle any core's causal diagonal).
- sigmoid stream starts after a short lead-in (K rows 0-512, V rows
  0-512, Q chunk 0); all remaining projections, stats, LN, and
  out-projection for chunks 0-2 run as PE/DVE filler inside the
  attention drive.
- ACT table discipline: stream is Sigmoid + Identity only; Silu (U
  gate) and Sqrt (LN stats) are batched into one mid-stream window.
- rope final add on DVE, V bias add on GpSimd (v5 had them swapped,
  coupling PE progress to the slow GpSimd queue).
- per-chunk stats/LN/out-projection drain: the serial tail is one
  256-row chunk instead of 512 rows.
"""
import numpy as np

import concourse.bacc as bacc
import concourse.tile as tile
from concourse import mybir
from concourse.bass_utils import run_bass_kernel_spmd

F32 = mybir.dt.float32
F32R = mybir.dt.float32r
BF16 = mybir.dt.bfloat16
AF = mybir.ActivationFunctionType
ALU = mybir.AluOpType

B, L, D, H, HD = 4, 2048, 1024, 16, 64
OWN = 1024          # rows owned per core
CQ = 256            # rows per chunk
PEXT = (4, 8, 12, 16)   # k-block extent per chunk (128-row k blocks)
SCALE = HD ** -0.5
LN_EPS = 1e-8
NCORES = 8

_CACHED = {}


def _build():
    nc = bacc.Bacc("TRN2", target_bir_lowering=False, debug=False)

    xkvT = nc.dram_tensor("xkvT", [D, L], BF16, kind="ExternalInput").ap()
    xqT = nc.dram_tensor("xqT", [D, OWN], BF16, kind="ExternalInput").ap()
    xq = nc.dram_tensor("xq", [OWN, D], F32, kind="ExternalInput").ap()
    wproj = nc.dram_tensor("wproj", [D, 4 * D], BF16, kind="ExternalInput").ap()
    wout = nc.dram_tensor("wout", [D, D], BF16, kind="ExternalInput").ap()
    cosk = nc.dram_tensor("cosk", [128, L], BF16, kind="ExternalInput").ap()
    sink = nc.dram_tensor("sink", [128, L], BF16, kind="ExternalInput").ap()
    cosq = nc.dram_tensor("cosq", [128, OWN], BF16, kind="ExternalInput").ap()
    sinq = nc.dram_tensor("sinq", [128, OWN], BF16, kind="ExternalInput").ap()
    p2 = nc.dram_tensor("p2", [128, 128], BF16, kind="ExternalInput").ap()
    maskD = nc.dram_tensor("maskD", [128, 8, 1024], BF16,
                           kind="ExternalInput").ap()
    bprojT = nc.dram_tensor("bprojT", [128, 32], F32, kind="ExternalInput").ap()
    gamT = nc.dram_tensor("gamT", [128, 8], F32R, kind="ExternalInput").ap()
    betT = nc.dram_tensor("betT", [128, 8], F32R, kind="ExternalInput").ap()
    ones128 = nc.dram_tensor("ones128", [128, 1], BF16,
                             kind="ExternalInput").ap()
    onesrowF = nc.dram_tensor("onesrowF", [1, 128], F32R,
                              kind="ExternalInput").ap()
    vbias = nc.dram_tensor("vbias", [128, D], BF16, kind="ExternalInput").ap()
    out = nc.dram_tensor("out", [OWN, D], F32, kind="ExternalOutput").ap()

    wp3 = wproj.rearrange("(t ki) n -> ki t n", ki=128)   # [128, 8, 4096]
    wo3 = wout.rearrange("(t ki) n -> ki t n", ki=128)    # [128, 8, 1024]
    xkv3 = xkvT.rearrange("(t ki) n -> ki t n", ki=128)   # [128, 8, 2048]
    xq3 = xqT.rearrange("(t ki) n -> ki t n", ki=128)     # [128, 8, 1024]

    with tile.TileContext(nc) as tc:
        with (
            tc.tile_pool(name="const", bufs=1) as cpool,
            tc.tile_pool(name="big", bufs=1) as big,
            tc.tile_pool(name="ph1x1", bufs=1) as ph1x1,
            tc.tile_pool(name="wring", bufs=2) as wring,
        ):
            trigp = tc.tile_pool(name="trig", bufs=1)
            ropep = tc.tile_pool(name="rope", bufs=2)
            trig = trigp.__enter__()
            rope = ropep.__enter__()
            qpoolp = tc.tile_pool(name="ph1a", bufs=1)
            qpool = qpoolp.__enter__()
            wring2p = tc.tile_pool(name="wring2", bufs=2)
            wring2 = wring2p.__enter__()
            ph1pool = tc.tile_pool(name="ph1", bufs=1)
            ph1 = ph1pool.__enter__()
            # ---- first-needed DMAs first ----
            wk0 = wring.tile([128, 8, 128], BF16, tag="wk")
            nc.sync.dma_start(wk0[:], wp3[:, :, 3 * D:3 * D + 128])
            p2sb = cpool.tile([128, 128], BF16)
            nc.sync.dma_start(p2sb[:], p2)
            bprojsb = cpool.tile([128, 32], F32)
            nc.sync.dma_start(bprojsb[:], bprojT)
            cksb0 = trig.tile([128, 1024], BF16, tag="cksb")
            nc.sync.dma_start(cksb0[:], cosk[:, 0:1024])
            sksb0 = trig.tile([128, 1024], BF16, tag="sksb")
            nc.sync.dma_start(sksb0[:], sink[:, 0:1024])
            xh0 = ph1.tile([128, 8, 1024], BF16, tag="xh")
            for t in range(8):
                nc.sync.dma_start(xh0[:, t, :], xkv3[:, t, 0:1024])
            # xqsb is a 2-entry ring of row halves: chunks 0,1 + U(r0)
            # use half 0; chunks 2,3 + U(r1) use half 1.
            xqsb0 = qpool.tile([128, 8, 512], BF16, tag="xqs")
            nc.sync.dma_start(xqsb0[:], xq3[:, :, 0:512])
            cqsb = qpool.tile([128, OWN], BF16)
            nc.sync.dma_start(cqsb[:], cosq)
            sqsb = qpool.tile([128, OWN], BF16)
            nc.sync.dma_start(sqsb[:], sinq)
            gamsb = cpool.tile([128, 8], F32R)
            nc.sync.dma_start(gamsb[:], gamT)
            betsb = cpool.tile([128, 8], F32R)
            nc.sync.dma_start(betsb[:], betT)
            o128 = cpool.tile([128, 1], BF16)
            nc.sync.dma_start(o128[:], ones128)
            orowF = cpool.tile([1, 128], F32R)
            nc.sync.dma_start(orowF[:], onesrowF)
            vbsb = cpool.tile([128, D], BF16)
            nc.sync.dma_start(vbsb[:], vbias)
            epsb = cpool.tile([1, 1], F32)
            nc.vector.memset(epsb[:], LN_EPS)
            xh1 = ph1x1.tile([128, 8, 1024], BF16, tag="xh1")
            nc.sync.dma_start(xh1[:], xkv3[:, :, 1024:2048])

            krot = big.tile([128, 8, L], BF16)      # K_rot^T
            v16 = big.tile([128, 16, D], BF16)      # V natural [row tiles]
            qrot = big.tile([128, 8, OWN], BF16)    # Q_rot^T
            silu16 = big.tile([128, 8, OWN], BF16)  # U^T raw, silu'd in bulk

            def rope_chain(psP, bias, cos_sl, sin_sl, dst, pspool, w):
                # dst(bf16) = (psP+b)*cos + rotate_half(psP+b)*sin
                t16 = rope.tile([128, 512], BF16, tag="t16")
                nc.scalar.activation(t16[:, 0:w], psP[:], AF.Identity,
                                     bias=bias)
                psR = pspool.tile([128, 512], F32, tag="psR", bufs=1)
                nc.tensor.matmul(psR[:, 0:w], p2sb[:], t16[:, 0:w],
                                 start=True, stop=True)
                tcos = rope.tile([128, 512], BF16, tag="tcos")
                nc.vector.scalar_tensor_tensor(
                    tcos[:, 0:w], psP[:], bias, cos_sl, ALU.add, ALU.mult)
                tsin = rope.tile([128, 512], BF16, tag="tsin")
                nc.vector.tensor_mul(tsin[:, 0:w], psR[:, 0:w], sin_sl)
                nc.gpsimd.tensor_add(dst, tcos[:, 0:w], tsin[:, 0:w])

            def k_gen(h, r, xh, cksb, sksb, pspool, first_wk=None):
                """K proj + rope for kb [4*(2h+r), 4*(2h+r)+4)."""
                for ct in range(8):
                    c0 = 3 * D + 128 * ct
                    if first_wk is not None and ct == 0:
                        wk = first_wk
                    else:
                        wk = wring.tile([128, 8, 128], BF16, tag="wk")
                        nc.sync.dma_start(wk[:], wp3[:, :, c0:c0 + 128])
                    ps = pspool.tile([128, 512], F32, tag="ps")
                    for t in range(8):
                        nc.tensor.matmul(
                            ps[:], wk[:, t, :],
                            xh[:, t, r * 512:(r + 1) * 512],
                            start=(t == 0), stop=(t == 7))
                    off = r * 512
                    rope_chain(ps, bprojsb[:, 24 + ct:25 + ct],
                               cksb[:, off:off + 512],
                               sksb[:, off:off + 512],
                               krot[:, ct, h * 1024 + off:
                                    h * 1024 + off + 512], pspool, 512)
                    yield

            def v_gen(h, rvs, xh, vpool):
                """V proj for row blocks grv = 8*h + rv, rv in rvs."""
                for vh in range(2):
                    v0 = D + 512 * vh
                    wvh = wring.tile([128, 8, 512], BF16, tag="wv", bufs=1)
                    nc.sync.dma_start(wvh[:], wp3[:, :, v0:v0 + 512])
                    for rv in rvs:
                        grv = h * 8 + rv
                        pv = vpool.tile([128, 512], F32, tag="ps")
                        for t in range(8):
                            nc.tensor.matmul(
                                pv[:], xh[:, t, 128 * rv:128 * (rv + 1)],
                                wvh[:, t, :], start=(t == 0), stop=(t == 7))
                        nc.vector.scalar_tensor_tensor(
                            v16[:, grv, 512 * vh:512 * (vh + 1)], pv[:], 0.0,
                            vbsb[:, 512 * vh:512 * (vh + 1)],
                            ALU.add, ALU.add)
                        yield

            def q_gen(c, xqs, pspool):
                """Q proj + rope for chunk c (256 rows)."""
                sl = slice(c * CQ, (c + 1) * CQ)          # global columns
                lsl = slice((c % 2) * CQ, (c % 2) * CQ + CQ)  # within half
                for ct in range(8):
                    q0 = 2 * D + 128 * ct
                    wq = wring2.tile([128, 8, 128], BF16, tag="wq")
                    nc.sync.dma_start(wq[:], wp3[:, :, q0:q0 + 128])
                    psq = pspool.tile([128, 512], F32, tag="ps")
                    for t in range(8):
                        nc.tensor.matmul(psq[:, 0:CQ], wq[:, t, :],
                                         xqs[:, t, lsl],
                                         start=(t == 0), stop=(t == 7))
                    rope_chain(psq[:, 0:CQ], bprojsb[:, 16 + ct:17 + ct],
                               cqsb[:, sl], sqsb[:, sl],
                               qrot[:, ct, sl], pspool, CQ)
                    yield

            def u_gen(r, xqs, pspool):
                """U proj for row half r; raw (+bias), Silu'd in bulk."""
                sl = slice(r * 512, (r + 1) * 512)
                for ct in range(8):
                    wu = wring2.tile([128, 8, 128], BF16, tag="wu")
                    nc.sync.dma_start(wu[:], wp3[:, :, 128 * ct:128 * (ct + 1)])
                    psu = pspool.tile([128, 512], F32, tag="ps")
                    for t in range(8):
                        nc.tensor.matmul(psu[:], wu[:, t, :],
                                         xqs[:, t, :],
                                         start=(t == 0), stop=(t == 7))
                    nc.scalar.activation(silu16[:, ct, sl], psu[:],
                                         AF.Identity,
                                         bias=bprojsb[:, ct:ct + 1])
                    yield

            # ---------- lead-in: K kb0-3, Q chunk 0, V rows 0-512 ----------
            with tc.tile_pool(name="ppj0", bufs=6, space="PSUM") as ppj0:
                for _ in k_gen(0, 0, xh0, cksb0, sksb0, ppj0, first_wk=wk0):
                    pass
                for _ in q_gen(0, xqsb0, ppj0):
                    pass
                for _ in v_gen(0, range(4), xh0, ppj0):
                    pass

            # ---------- attention stream ----------
            with (
                tc.tile_pool(name="ph2", bufs=1, side="right") as ph2,
                tc.tile_pool(name="aring", bufs=2, side="right") as aring,
            ):
                attn_scope = [tc.tile_pool(name="psS_", bufs=2, space="PSUM"),
                              tc.tile_pool(name="psO_", bufs=1, space="PSUM")]
                psSp, psOp = [p.__enter__() for p in attn_scope]
                attnT = ph2.tile([128, 8, OWN], BF16)
                msb01 = ph2.tile([128, 4, 1024], BF16)
                nc.sync.dma_start(msb01[:], maskD[:, 0:4, :])
                late = {}

                def chain_gen(c, hp):
                    p = PEXT[c]
                    qA = qrot[0:64, hp, c * CQ:(c + 1) * CQ]
                    qB = qrot[64:128, hp, c * CQ:(c + 1) * CQ]
                    with tc.high_priority():
                        psO = psOp.tile([128, CQ], F32, tag="psO")
                    for g in range(p // 2):
                        with tc.high_priority():
                            psS = psSp.tile([128, 1024], F32, tag="psS")
                            for j in range(2):
                                kb = 2 * g + j
                                nc.tensor.matmul(
                                    psS[:, j * CQ:(j + 1) * CQ],
                                    krot[0:64, hp, 128 * kb:128 * (kb + 1)],
                                    qA, start=True, stop=True)
                                nc.tensor.matmul(
                                    psS[:, 512 + j * CQ:512 + (j + 1) * CQ],
                                    krot[64:128, hp, 128 * kb:128 * (kb + 1)],
                                    qB, start=True, stop=True)
                            aAB = aring.tile([128, 1024], BF16, tag="aAB")
                            nc.scalar.activation(aAB[:], psS[:], AF.Sigmoid,
                                                 scale=SCALE)
                            if g >= p // 2 - 2:
                                gg = g - (p // 2 - 2)
                                if c < 2:
                                    mt = msb01[:, 2 * c + gg, :]
                                else:
                                    mt = late["msb23"][:, 2 * (c - 2) + gg, :]
                                nc.vector.tensor_mul(aAB[:], aAB[:], mt)
                            for j in range(2):
                                kb = 2 * g + j
                                first = (g == 0 and j == 0)
                                last = (g == p // 2 - 1 and j == 1)
                                nc.tensor.matmul(
                                    psO[0:64, :],
                                    v16[:, kb, 128 * hp:128 * hp + 64],
                                    aAB[:, j * CQ:(j + 1) * CQ],
                                    start=first, stop=last,
                                    tile_position=(0, 0))
                                nc.tensor.matmul(
                                    psO[64:128, :],
                                    v16[:, kb, 128 * hp + 64:128 * (hp + 1)],
                                    aAB[:, 512 + j * CQ:512 + (j + 1) * CQ],
                                    start=first, stop=last,
                                    tile_position=(0, 64))
                        yield
                    with tc.high_priority():
                        nc.vector.tensor_copy(
                            attnT[:, hp, c * CQ:(c + 1) * CQ], psO[:])

                def stats_acc_gen(c):
                    csl = slice(c * CQ, (c + 1) * CQ)
                    stage = late["stage"]
                    with tc.tile_pool(name=f"psT{c}", bufs=2,
                                      space="PSUM") as psTp:
                        psSum = psTp.tile([1, CQ], F32, tag="st")
                        psSq = psTp.tile([1, CQ], F32, tag="st")
                        for hp in range(8):
                            sq = late["sqring"].tile([128, CQ], BF16,
                                                     tag="sq")
                            nc.vector.tensor_mul(sq[:], attnT[:, hp, csl],
                                                 attnT[:, hp, csl])
                            nc.tensor.matmul(psSum[:], o128[:],
                                             attnT[:, hp, csl],
                                             start=(hp == 0), stop=(hp == 7))
                            nc.tensor.matmul(psSq[:], o128[:], sq[:],
                                             start=(hp == 0), stop=(hp == 7))
                            yield
                        nc.vector.tensor_scalar_mul(
                            stage[0:1, c, 0, :], psSum[:], 1.0 / D)
                        nc.vector.tensor_scalar_mul(
                            stage[0:1, c, 1, :], psSq[:], 1.0 / D)
                        yield

                def sqrt_finish(cs):
                    """DVE variance prep, adjacent ACT Sqrts, DVE finish.
                    stage[c,0]: mu -> -mu*rstd;  stage[c,1]: m2 -> rstd."""
                    stage = late["stage"]
                    for c in cs:
                        musq = late["sqring"].tile([1, CQ], F32R, tag="musq")
                        nc.vector.tensor_mul(musq[:], stage[0:1, c, 0, :],
                                             stage[0:1, c, 0, :])
                        nc.vector.tensor_sub(stage[0:1, c, 1, :],
                                             stage[0:1, c, 1, :], musq[:])
                    for c in cs:
                        nc.scalar.activation(stage[0:1, c, 1, :],
                                             stage[0:1, c, 1, :], AF.Sqrt,
                                             bias=epsb[:])
                    for c in cs:
                        with nc.allow_low_precision("f32r rstd for matmul"):
                            nc.vector.reciprocal(stage[0:1, c, 1, :],
                                                 stage[0:1, c, 1, :])
                        nc.vector.tensor_mul(stage[0:1, c, 0, :],
                                             stage[0:1, c, 0, :],
                                             stage[0:1, c, 1, :])
                        nc.vector.tensor_scalar_mul(stage[0:1, c, 0, :],
                                                    stage[0:1, c, 0, :], -1.0)

                def ln_gen(c):
                    csl = slice(c * CQ, (c + 1) * CQ)
                    stage, bcast, gring = (late["stage"], late["bcast"],
                                           late["gring"])
                    with tc.tile_pool(name=f"psG{c}", bufs=2,
                                      space="PSUM") as psGp:
                        psRb = psGp.tile([128, CQ], F32, tag="bc")
                        nc.tensor.matmul(psRb[:], orowF[0:1, :],
                                         stage[0:1, c, 1, :],
                                         start=True, stop=True)
                        rstd_b = bcast.tile([128, CQ], BF16, tag="rb")
                        nc.scalar.activation(rstd_b[:], psRb[:], AF.Copy)
                        psNb = psGp.tile([128, CQ], F32, tag="bc")
                        nc.tensor.matmul(psNb[:], orowF[0:1, :],
                                         stage[0:1, c, 0, :],
                                         start=True, stop=True)
                        nmr_b = bcast.tile([128, CQ], BF16, tag="nb")
                        nc.scalar.activation(nmr_b[:], psNb[:], AF.Copy)
                        yield

                        # gated = ((aT*gam)*rstd_b + (nmr_b*gam+bet))*silu
                        for ct in range(8):
                            g1 = gring.tile([128, CQ], BF16, tag="g1")
                            nc.vector.scalar_tensor_tensor(
                                g1[:], attnT[:, ct, csl],
                                gamsb[:, ct:ct + 1],
                                rstd_b[:], ALU.mult, ALU.mult)
                            g2 = gring.tile([128, CQ], BF16, tag="g2")
                            nc.vector.scalar_tensor_tensor(
                                g2[:], nmr_b[:], gamsb[:, ct:ct + 1], g1[:],
                                ALU.mult, ALU.add)
                            nc.vector.scalar_tensor_tensor(
                                attnT[:, ct, csl], g2[:],
                                betsb[:, ct:ct + 1],
                                silu16[:, ct, csl],
                                ALU.add, ALU.mult)
                            yield

                def outproj_gen(c):
                    oring, woring = late["oring"], late["woring"]
                    if "wo0" not in late:
                        late["wo0"] = woring.tile([128, 8, 512], BF16,
                                                  tag="wo0", name="wo0")
                        nc.sync.dma_start(late["wo0"][:], wo3[:, :, 0:512])
                        late["wo1"] = woring.tile([128, 8, 512], BF16,
                                                  tag="wo1", name="wo1")
                        nc.sync.dma_start(late["wo1"][:], wo3[:, :, 512:D])
                    wos = (late["wo0"], late["wo1"])
                    with tc.tile_pool(name=f"psP{c}", bufs=2,
                                      space="PSUM") as psPp:
                        for rw in range(2 * c, 2 * c + 2):
                            r0 = 128 * rw
                            xqn = oring.tile([128, D], F32, tag="xqn",
                                             bufs=1)
                            nc.sync.dma_start(xqn[:], xq[r0:r0 + 128, :])
                            for oh in range(2):
                                psOut = psPp.tile([128, 512], F32, tag="po")
                                for ct in range(8):
                                    st = attnT[:, ct, r0:r0 + 128]
                                    nc.tensor.matmul(
                                        psOut[:], st, wos[oh][:, ct, :],
                                        start=(ct == 0), stop=(ct == 7))
                                    if ct == 3:
                                        yield
                                osb = oring.tile([128, 512], F32, tag="osb")
                                nc.vector.tensor_add(
                                    osb[:], psOut[:],
                                    xqn[:, 512 * oh:512 * (oh + 1)])
                                nc.sync.dma_start(
                                    out[r0:r0 + 128, 512 * oh:512 * (oh + 1)],
                                    osb[:])
                                yield

                def mega_filler():
                    cks1 = trig.tile([128, 1024], BF16, tag="cksb")
                    nc.sync.dma_start(cks1[:], cosk[:, 1024:2048])
                    sks1 = trig.tile([128, 1024], BF16, tag="sksb")
                    nc.sync.dma_start(sks1[:], sink[:, 1024:2048])
                    with tc.tile_pool(name="pfil", bufs=2,
                                      space="PSUM") as pfil:
                        yield from k_gen(0, 1, xh0, cksb0, sksb0, pfil)
                        yield from v_gen(0, range(4, 8), xh0, pfil)
                        ph1pool.__exit__(None, None, None)  # xh0 done
                        yield from q_gen(1, xqsb0, pfil)
                        yield from u_gen(0, xqsb0, pfil)
                        xqsb1 = qpool.tile([128, 8, 512], BF16, tag="xqs")
                        nc.sync.dma_start(xqsb1[:], xq3[:, :, 512:1024])
                        yield from q_gen(2, xqsb1, pfil)
                        yield from q_gen(3, xqsb1, pfil)
                        yield from k_gen(1, 0, xh1, cks1, sks1, pfil)
                        yield from v_gen(1, range(4), xh1, pfil)
                        m23p = tc.tile_pool(name="msb23", bufs=1,
                                            side="right")
                        late["msb23_pool"] = m23p
                        m23 = m23p.__enter__()
                        late["msb23"] = m23.tile([128, 4, 1024], BF16,
                                                 name="msb23")
                        nc.sync.dma_start(late["msb23"][:], maskD[:, 4:8, :])
                        yield from k_gen(1, 1, xh1, cks1, sks1, pfil)
                        yield from v_gen(1, range(4, 8), xh1, pfil)
                        yield from u_gen(1, xqsb1, pfil)
                    # phase-1 SBUF no longer needed; make room for tail pools
                    wring2p.__exit__(None, None, None)
                    qpoolp.__exit__(None, None, None)
                    ropep.__exit__(None, None, None)
                    trigp.__exit__(None, None, None)
                    for nm, bufs in (("sqring", 1), ("bcast", 1),
                                     ("gring", 1), ("oring", 2),
                                     ("woring", 1), ("statp", 1)):
                        pp = tc.tile_pool(name=nm, bufs=bufs, side="right")
                        late[nm + "_pool"] = pp
                        late[nm] = pp.__enter__()
                    late["stage"] = late["statp"].tile([1, 4, 2, CQ], F32R,
                                                       name="stage")
                    yield
                    # only chunks 0,1 here: their chains are fully EMITTED
                    # by now (emission-time dataflow — a read emitted before
                    # its writer would bind to the stale tile version).
                    for c in range(2):
                        yield from stats_acc_gen(c)
                    # one ACT table window: Silu (U gate) + 2 Sqrts
                    nc.scalar.activation(silu16[:], silu16[:], AF.Silu)
                    sqrt_finish(range(2))
                    yield
                    for c in range(2):
                        yield from ln_gen(c)
                        yield from outproj_gen(c)

                def drive(chains, fillers, steps_per=1):
                    fi = iter(fillers)
                    cur = None
                    for ch in chains:
                        for _ in ch:
                            for _ in range(steps_per):
                                if cur is None:
                                    cur = next(fi, None)
                                    if cur is None:
                                        break
                                if next(cur, StopIteration) is StopIteration:
                                    cur = None
                    while True:
                        if cur is None:
                            cur = next(fi, None)
                            if cur is None:
                                break
                        if next(cur, StopIteration) is StopIteration:
                            cur = None

                drive([chain_gen(c, hp) for c in range(4) for hp in range(8)],
                      [mega_filler()], steps_per=2)

                # ---- tail: chunks 2,3 stats/LN/out-projection (chunk 2
                # work overlaps chunk 3's still-running stream) ----
                for _ in stats_acc_gen(2):
                    pass
                for _ in stats_acc_gen(3):
                    pass
                sqrt_finish([2, 3])
                for _ in ln_gen(2):
                    pass
                for _ in outproj_gen(2):
                    pass
                for _ in ln_gen(3):
                    pass
                for _ in outproj_gen(3):
                    pass
                for p in reversed(attn_scope):
                    p.__exit__(None, None, None)
                for nm in ("statp", "woring", "oring", "gring", "bcast",
                           "sqring", "msb23"):
                    late[nm + "_pool"].__exit__(None, None, None)
    nc.finalize()
    return nc


# chunk order per core group g (256-row blocks of the 2048 sequence):
# sorted by causal extent; both cores fit PEXT = (4, 8, 12, 16).
_BLOCKS = {0: (0, 3, 4, 7), 1: (1, 2, 5, 6)}


def _own_rows(g):
    return np.concatenate([np.arange(256 * b, 256 * b + 256)
                           for b in _BLOCKS[g]])


def _host_prep(x, attn_mask, W_proj, b_proj, ln_gamma, ln_beta, W_out, b_out):
    """Build the 8 per-core input maps."""
    import ml_dtypes
    bf16 = ml_dtypes.bfloat16

    x = np.asarray(x, dtype=np.float32)
    attn_mask = np.asarray(attn_mask)
    W_proj = np.ascontiguousarray(np.asarray(W_proj, dtype=np.float32))
    W_out = np.ascontiguousarray(np.asarray(W_out, dtype=np.float32))
    b_proj = np.asarray(b_proj, dtype=np.float32)
    b_out = np.asarray(b_out, dtype=np.float32)
    ln_gamma = np.asarray(ln_gamma, dtype=np.float32)
    ln_beta = np.asarray(ln_beta, dtype=np.float32)

    inv = 1.0 / (10000.0 ** (np.arange(0, HD, 2, dtype=np.float64) / HD))
    ang = np.outer(inv, np.arange(L, dtype=np.float64))       # [32, L]
    c64 = np.concatenate([np.cos(ang), np.cos(ang)], 0)
    s64 = np.concatenate([np.sin(ang), np.sin(ang)], 0)
    cosk = np.concatenate([c64, c64], 0).astype(np.float32)   # [128, L]
    sink = np.concatenate([s64, s64], 0).astype(np.float32)

    p2 = np.zeros((128, 128), dtype=np.float32)
    for base in (0, 64):
        for m in range(32):
            p2[base + m + 32, base + m] = -1.0
        for m in range(32, 64):
            p2[base + m - 32, base + m] = 1.0

    shared = dict(
        wproj=W_proj.astype(bf16), wout=W_out.astype(bf16),
        cosk=cosk.astype(bf16), sink=sink.astype(bf16),
        p2=p2.astype(bf16),
        bprojT=np.ascontiguousarray(b_proj.reshape(32, 128).T),
        gamT=np.ascontiguousarray(ln_gamma.reshape(8, 128).T),
        betT=np.ascontiguousarray(ln_beta.reshape(8, 128).T),
        ones128=np.ones((128, 1), np.float32).astype(bf16),
        onesrowF=np.ones((1, 128), np.float32),
        vbias=np.broadcast_to(b_proj[D:2 * D], (128, D)).astype(bf16),
    )

    in_maps = []
    for cid in range(NCORES):
        b, g = divmod(cid, 2)
        own = _own_rows(g)
        xb = x[b]
        xqc = np.ascontiguousarray(xb[own])
        m = dict(shared)
        m["xkvT"] = np.ascontiguousarray(xb.T).astype(bf16)
        m["xqT"] = np.ascontiguousarray(xqc.T).astype(bf16)
        m["xq"] = xqc + b_out[None, :]
        m["cosq"] = np.ascontiguousarray(cosk[:, own]).astype(bf16)
        m["sinq"] = np.ascontiguousarray(sink[:, own]).astype(bf16)
        # masks: for chunk c, groups cover kb PEXT[c]-4 .. PEXT[c]-1,
        # duplicated for the two heads of a head-pair:
        # cols [0:256]=kb_a, [256:512]=kb_a+1, [512:1024]=same again
        am = attn_mask[b]
        md = np.zeros((8, 128, 1024), dtype=np.float32)
        for c in range(4):
            qg = own[c * CQ:(c + 1) * CQ]
            for gg in range(2):
                for j in range(2):
                    kb = PEXT[c] - 4 + 2 * gg + j
                    blk = am[qg][:, 128 * kb:128 * (kb + 1)].T  # [128,256]
                    md[2 * c + gg, :, 256 * j:256 * (j + 1)] = blk
                    md[2 * c + gg, :, 512 + 256 * j:512 + 256 * (j + 1)] = blk
        m["maskD"] = np.ascontiguousarray(
            md.transpose(1, 0, 2).astype(bf16))
        in_maps.append(m)
    return in_maps


def kernel(**inputs):
    if "nc" not in _CACHED:
        _CACHED["nc"] = _build()
    nc = _CACHED["nc"]
    in_maps = _host_prep(**inputs)
    res = run_bass_kernel_spmd(nc, in_maps, list(range(NCORES)))
    full = np.empty((B, L, D), dtype=np.float32)
    for cid in range(NCORES):
        b, g = divmod(cid, 2)
        o = res.results[cid]["out"]
        own = _own_rows(g)
        full[b, own] = o
    return full


# revision 46
# speedup vs baseline: 1.0808x; 1.0808x over previous
"""HSTU block kernel v4 for 8 TRN2 NeuronCores (nn_HSTU_66279935312625).

Sharding: 2 cores per batch (B=4). Core pair splits the 2048 rows
causally-balanced: core g=0 owns rows [0,512)+[1536,2048), g=1 owns
[512,1536). Every core recomputes K/V projections for all 2048 rows of
its batch (communication-free). The program is identical on all cores;
all per-core variation lives in host-prepared input data.

v4: generator-driven schedule. All matmul operands bf16 (PE weight
double-buffering), one [128,1024] sigmoid per 2-head kb step.
Emission order interleaves at kb-step granularity:
  phase1: K(h0) V(h0) K(h1) Q+U          (PE-bound, rope on Act/DVE/GpSimd)
  phase2a: attn(u0) chains with V(h1) groups as PE filler
  phase2b: attn(u1) chains with u0 stats/LN/out-proj as PE filler
  phase2c: u1 tail
Act sigmoid stream stays saturated; PE fills its slack with tail work.
"""
import numpy as np

import concourse.bacc as bacc
import concourse.tile as tile
from concourse import mybir
from concourse.bass_utils import run_bass_kernel_spmd

F32 = mybir.dt.float32
F32R = mybir.dt.float32r
BF16 = mybir.dt.bfloat16
AF = mybir.ActivationFunctionType
ALU = mybir.AluOpType

B, L, D, H, HD = 4, 2048, 1024, 16, 64
OWN = 1024          # rows owned per core
UQ = 512            # rows per q-unit
EXT = (8, 16)       # k-block extent per q-unit (128-row k blocks)
SCALE = HD ** -0.5
LN_EPS = 1e-8
NCORES = 8

_CACHED = {}


def _build():
    nc = bacc.Bacc("TRN2", target_bir_lowering=False, debug=False)

    xkvT = nc.dram_tensor("xkvT", [D, L], BF16, kind="ExternalInput").ap()
    xqT = nc.dram_tensor("xqT", [D, OWN], BF16, kind="ExternalInput").ap()
    xq = nc.dram_tensor("xq", [OWN, D], F32, kind="ExternalInput").ap()
    wproj = nc.dram_tensor("wproj", [D, 4 * D], BF16, kind="ExternalInput").ap()
    wout = nc.dram_tensor("wout", [D, D], BF16, kind="ExternalInput").ap()
    cosk = nc.dram_tensor("cosk", [128, L], BF16, kind="ExternalInput").ap()
    sink = nc.dram_tensor("sink", [128, L], BF16, kind="ExternalInput").ap()
    cosq = nc.dram_tensor("cosq", [128, OWN], BF16, kind="ExternalInput").ap()
    sinq = nc.dram_tensor("sinq", [128, OWN], BF16, kind="ExternalInput").ap()
    p2 = nc.dram_tensor("p2", [128, 128], BF16, kind="ExternalInput").ap()
    maskT = nc.dram_tensor("maskT", [128, 16, UQ], BF16,
                           kind="ExternalInput").ap()
    bprojT = nc.dram_tensor("bprojT", [128, 32], F32, kind="ExternalInput").ap()
    gamT = nc.dram_tensor("gamT", [128, 8], F32R, kind="ExternalInput").ap()
    betT = nc.dram_tensor("betT", [128, 8], F32R, kind="ExternalInput").ap()
    bcol = nc.dram_tensor("bcol", [1, 4 * D], BF16, kind="ExternalInput").ap()
    ones128 = nc.dram_tensor("ones128", [128, 1], BF16,
                             kind="ExternalInput").ap()
    onesrow = nc.dram_tensor("onesrow", [1, UQ], BF16,
                             kind="ExternalInput").ap()
    onesrowF = nc.dram_tensor("onesrowF", [1, 128], F32R,
                              kind="ExternalInput").ap()
    vbias = nc.dram_tensor("vbias", [128, D], BF16, kind="ExternalInput").ap()
    out = nc.dram_tensor("out", [OWN, D], F32, kind="ExternalOutput").ap()

    wp3 = wproj.rearrange("(t ki) n -> ki t n", ki=128)   # [128, 8, 4096]
    wo3 = wout.rearrange("(t ki) n -> ki t n", ki=128)    # [128, 8, 1024]
    xkv3 = xkvT.rearrange("(t ki) n -> ki t n", ki=128)   # [128, 8, 2048]
    xq3 = xqT.rearrange("(t ki) n -> ki t n", ki=128)     # [128, 8, 1024]

    with tile.TileContext(nc) as tc:
        with (
            tc.tile_pool(name="const", bufs=1) as cpool,
            tc.tile_pool(name="big", bufs=1) as big,
            tc.tile_pool(name="ph1x1", bufs=1) as ph1x1,
            tc.tile_pool(name="wring", bufs=2) as wring,
        ):
            ph1scope = [tc.tile_pool(name="rope", bufs=3),
                        tc.tile_pool(name="trig", bufs=1),
                        tc.tile_pool(name="ppj", bufs=6, space="PSUM"),
                        tc.tile_pool(name="prt", bufs=2, space="PSUM")]
            rope, trig, ppj, prt = [p.__enter__() for p in ph1scope]
            ph1pool = tc.tile_pool(name="ph1", bufs=1)
            ph1 = ph1pool.__enter__()
            # ---- first-needed DMAs first ----
            wk0 = wring.tile([128, 8, 128], BF16, tag="wk")
            nc.sync.dma_start(wk0[:], wp3[:, :, 3 * D:3 * D + 128])
            p2sb = cpool.tile([128, 128], BF16)
            nc.sync.dma_start(p2sb[:], p2)
            bprojsb = cpool.tile([128, 32], F32)
            nc.sync.dma_start(bprojsb[:], bprojT)
            cksb0 = trig.tile([128, 1024], BF16, tag="cksb")
            nc.sync.dma_start(cksb0[:], cosk[:, 0:1024])
            sksb0 = trig.tile([128, 1024], BF16, tag="sksb")
            nc.sync.dma_start(sksb0[:], sink[:, 0:1024])
            xh0 = ph1.tile([128, 8, 1024], BF16, tag="xh")
            for t in range(8):
                nc.sync.dma_start(xh0[:, t, :], xkv3[:, t, 0:1024])
            gamsb = cpool.tile([128, 8], F32R)
            nc.sync.dma_start(gamsb[:], gamT)
            betsb = cpool.tile([128, 8], F32R)
            nc.sync.dma_start(betsb[:], betT)
            o128 = cpool.tile([128, 1], BF16)
            nc.sync.dma_start(o128[:], ones128)
            orow = cpool.tile([1, UQ], BF16)
            nc.sync.dma_start(orow[:], onesrow)
            orowF = cpool.tile([1, 128], F32R)
            nc.sync.dma_start(orowF[:], onesrowF)
            vbsb = cpool.tile([128, D], BF16)
            nc.sync.dma_start(vbsb[:], vbias)
            epsb = cpool.tile([1, 1], F32)
            nc.vector.memset(epsb[:], LN_EPS)

            krot = big.tile([128, 8, L], BF16)      # K_rot^T
            v16 = big.tile([128, 16, D], BF16)      # V natural [row tiles]
            qrot = big.tile([128, 8, OWN], BF16)    # Q_rot^T
            silu16 = big.tile([128, 8, OWN], BF16)  # silu(U)^T

            def rope_chain(psP, bias, cos_sl, sin_sl, dst):
                # dst(bf16) = (psP+b)*cos + rotate_half(psP+b)*sin
                t16 = rope.tile([128, UQ], BF16, tag="t16")
                nc.scalar.activation(t16[:], psP[:], AF.Identity, bias=bias)
                psR = prt.tile([128, UQ], F32, tag="psR")
                nc.tensor.matmul(psR[:], p2sb[:], t16[:], start=True,
                                 stop=True)
                tcos = rope.tile([128, UQ], F32, tag="tcos")
                nc.vector.scalar_tensor_tensor(
                    tcos[:], psP[:], bias, cos_sl, ALU.add, ALU.mult)
                tsin = rope.tile([128, UQ], F32, tag="tsin")
                nc.vector.tensor_mul(tsin[:], psR[:], sin_sl)
                nc.gpsimd.tensor_add(dst, tcos[:], tsin[:])

            def k_block(h, xh, cksb, sksb, first_wk=None):
                for ct in range(8):
                    c0 = 3 * D + 128 * ct
                    if first_wk is not None and ct == 0:
                        wk = first_wk
                    else:
                        wk = wring.tile([128, 8, 128], BF16, tag="wk")
                        nc.sync.dma_start(wk[:], wp3[:, :, c0:c0 + 128])
                    for r in range(2):
                        ps = ppj.tile([128, UQ], F32, tag="ps")
                        for t in range(8):
                            nc.tensor.matmul(
                                ps[:], wk[:, t, :],
                                xh[:, t, r * UQ:(r + 1) * UQ],
                                start=(t == 0), stop=(t == 7))
                        off = r * UQ
                        rope_chain(ps, bprojsb[:, 24 + ct:25 + ct],
                                   cksb[:, off:off + UQ],
                                   sksb[:, off:off + UQ],
                                   krot[:, ct, h * 1024 + off:
                                        h * 1024 + off + UQ])

            def v_group_gen(h, xh, vpool):
                for vh in range(2):
                    v0 = D + UQ * vh
                    wvh = wring.tile([128, 8, UQ], BF16, tag="wv", bufs=1)
                    nc.sync.dma_start(wvh[:], wp3[:, :, v0:v0 + UQ])
                    for rv in range(8):
                        grv = h * 8 + rv
                        pv = vpool.tile([128, UQ], F32, tag="ps")
                        for t in range(8):
                            nc.tensor.matmul(
                                pv[:], xh[:, t, 128 * rv:128 * (rv + 1)],
                                wvh[:, t, :], start=(t == 0), stop=(t == 7))
                            if t == 3:
                                yield
                        nc.vector.scalar_tensor_tensor(
                            v16[:, grv, UQ * vh:UQ * (vh + 1)], pv[:], 0.0,
                            vbsb[:, UQ * vh:UQ * (vh + 1)],
                            ALU.add, ALU.add)
                        yield

            # ---------- phase 1: K(h0), V(h0), K(h1), Q+U ----------
            xh1 = ph1x1.tile([128, 8, 1024], BF16, tag="xh1")
            nc.sync.dma_start(xh1[:], xkv3[:, :, 1024:2048])
            k_block(0, xh0, cksb0, sksb0, first_wk=wk0)
            for _ in v_group_gen(0, xh0, ppj):
                pass
            ph1pool.__exit__(None, None, None)
            cksb1 = trig.tile([128, 1024], BF16, tag="cksb")
            nc.sync.dma_start(cksb1[:], cosk[:, 1024:2048])
            sksb1 = trig.tile([128, 1024], BF16, tag="sksb")
            nc.sync.dma_start(sksb1[:], sink[:, 1024:2048])
            k_block(1, xh1, cksb1, sksb1)

            with (
                tc.tile_pool(name="ph1a", bufs=1) as ph1a,
                tc.tile_pool(name="wring2", bufs=2) as wring2,
            ):
                xqsb = ph1a.tile([128, 8, OWN], BF16)
                nc.sync.dma_start(xqsb[:], xq3)
                cqsb = ph1a.tile([128, OWN], BF16)
                nc.sync.dma_start(cqsb[:], cosq)
                sqsb = ph1a.tile([128, OWN], BF16)
                nc.sync.dma_start(sqsb[:], sinq)

                for ct in range(8):
                    wu = wring2.tile([128, 8, 128], BF16, tag="wu")
                    nc.sync.dma_start(wu[:], wp3[:, :, 128 * ct:128 * (ct + 1)])
                    q0 = 2 * D + 128 * ct
                    wq = wring2.tile([128, 8, 128], BF16, tag="wq")
                    nc.sync.dma_start(wq[:], wp3[:, :, q0:q0 + 128])
                    for r in range(2):
                        sl = slice(r * UQ, (r + 1) * UQ)
                        psu = ppj.tile([128, UQ], F32, tag="ps")
                        for t in range(8):
                            nc.tensor.matmul(psu[:], wu[:, t, :],
                                             xqsb[:, t, sl],
                                             start=(t == 0), stop=(t == 7))
                        nc.scalar.activation(silu16[:, ct, sl], psu[:],
                                             AF.Silu,
                                             bias=bprojsb[:, ct:ct + 1])
                        psq = ppj.tile([128, UQ], F32, tag="ps")
                        for t in range(8):
                            nc.tensor.matmul(psq[:], wq[:, t, :],
                                             xqsb[:, t, sl],
                                             start=(t == 0), stop=(t == 7))
                        rope_chain(psq, bprojsb[:, 16 + ct:17 + ct],
                                   cqsb[:, sl], sqsb[:, sl],
                                   qrot[:, ct, sl])

            for p in reversed(ph1scope):
                p.__exit__(None, None, None)

            # ---------- phase 2 ----------
            with (
                tc.tile_pool(name="ph2", bufs=1) as ph2,
                tc.tile_pool(name="aring", bufs=3) as aring,
                tc.tile_pool(name="sqring", bufs=1) as sqring,
                tc.tile_pool(name="bcast", bufs=1) as bcast,
                tc.tile_pool(name="gring", bufs=1) as gring,
                tc.tile_pool(name="oring", bufs=2) as oring,
                tc.tile_pool(name="woring", bufs=1) as woring,
            ):
                attn_scope = [tc.tile_pool(name="psS_", bufs=2, space="PSUM"),
                              tc.tile_pool(name="psO_", bufs=2, space="PSUM")]
                psSp, psOp = [p.__enter__() for p in attn_scope]
                attnT = ph2.tile([128, 8, L // 2], BF16)
                statr = ph2.tile([1, 4, UQ], F32R)
                tail_sb = {}

                def attn_chain_gen(u, hp):
                    psO = psOp.tile([128, UQ], F32, tag="psO")
                    qA = qrot[0:64, hp, u * UQ:(u + 1) * UQ]
                    qB = qrot[64:128, hp, u * UQ:(u + 1) * UQ]
                    kbs = range(EXT[u] - 1, -1, -1) if u == 1 else \
                        range(EXT[u])
                    for j, kb in enumerate(kbs):
                        psS = psSp.tile([128, 1024], F32, tag="psS")
                        nc.tensor.matmul(
                            psS[:, 0:UQ],
                            krot[0:64, hp, 128 * kb:128 * (kb + 1)],
                            qA, start=True, stop=True)
                        nc.tensor.matmul(
                            psS[:, UQ:1024],
                            krot[64:128, hp, 128 * kb:128 * (kb + 1)],
                            qB, start=True, stop=True)
                        aAB = aring.tile([128, 1024], BF16, tag="aAB")
                        nc.scalar.activation(aAB[:], psS[:], AF.Sigmoid,
                                             scale=SCALE)
                        mi = kb - 8 * u
                        if 0 <= mi < 8:
                            mj = 8 * u + mi
                            nc.vector.tensor_mul(aAB[:, 0:UQ], aAB[:, 0:UQ],
                                                 msb[:, mj, :])
                            nc.vector.tensor_mul(aAB[:, UQ:1024],
                                                 aAB[:, UQ:1024],
                                                 msb[:, mj, :])
                        last = j == EXT[u] - 1
                        nc.tensor.matmul(
                            psO[0:64, :],
                            v16[:, kb, 128 * hp:128 * hp + 64],
                            aAB[:, 0:UQ], start=(j == 0), stop=last,
                            tile_position=(0, 0))
                        nc.tensor.matmul(
                            psO[64:128, :],
                            v16[:, kb, 128 * hp + 64:128 * (hp + 1)],
                            aAB[:, UQ:1024], start=(j == 0), stop=last,
                            tile_position=(0, 64))
                        yield
                    nc.vector.tensor_copy(
                        attnT[:, hp, u * UQ:(u + 1) * UQ], psO[:])

                def stats_gen(u):
                    usl = slice(u * UQ, (u + 1) * UQ)
                    with tc.tile_pool(name=f"psT{u}", bufs=2,
                                      space="PSUM") as psTp:
                        psSum = psTp.tile([1, UQ], F32, tag="st")
                        psSq = psTp.tile([1, UQ], F32, tag="st")
                        for hp in range(8):
                            sq = sqring.tile([128, UQ], BF16, tag="sq")
                            nc.vector.tensor_mul(sq[:], attnT[:, hp, usl],
                                                 attnT[:, hp, usl])
                            nc.tensor.matmul(psSum[:], o128[:],
                                             attnT[:, hp, usl],
                                             start=(hp == 0), stop=(hp == 7))
                            nc.tensor.matmul(psSq[:], o128[:], sq[:],
                                             start=(hp == 0), stop=(hp == 7))
                            yield

                        mu = statr[0:1, 0, :]
                        nc.vector.tensor_scalar_mul(mu, psSum[:], 1.0 / D)
                        m2 = statr[0:1, 1, :]
                        nc.vector.tensor_scalar_mul(m2, psSq[:], 1.0 / D)
                        musq = statr[0:1, 2, :]
                        nc.vector.tensor_mul(musq, mu, mu)
                        varr = statr[0:1, 1, :]
                        nc.vector.tensor_sub(varr, m2, musq)
                        rstd = statr[0:1, 3, :]
                        nc.scalar.activation(rstd, varr, AF.Sqrt, bias=epsb[:])
                        with nc.allow_low_precision("f32r rstd for matmul"):
                            nc.vector.reciprocal(rstd, rstd)
                        nmr = statr[0:1, 2, :]
                        nc.vector.tensor_mul(nmr, mu, rstd)
                        nc.vector.tensor_scalar_mul(nmr, nmr, -1.0)
                        yield

                def ln_gen(u):
                    usl = slice(u * UQ, (u + 1) * UQ)
                    with tc.tile_pool(name=f"psG{u}", bufs=2,
                                      space="PSUM") as psGp:
                        psRb = psGp.tile([128, UQ], F32, tag="bc")
                        nc.tensor.matmul(psRb[:], orowF[0:1, :],
                                         statr[0:1, 3, :],
                                         start=True, stop=True)
                        rstd_b = bcast.tile([128, UQ], BF16, tag="rb")
                        nc.scalar.activation(rstd_b[:], psRb[:], AF.Copy)
                        psNb = psGp.tile([128, UQ], F32, tag="bc")
                        nc.tensor.matmul(psNb[:], orowF[0:1, :],
                                         statr[0:1, 2, :],
                                         start=True, stop=True)
                        nmr_b = bcast.tile([128, UQ], BF16, tag="nb")
                        nc.scalar.activation(nmr_b[:], psNb[:], AF.Copy)
                        yield

                        # gated = ((aT*gam)*rstd_b + (nmr_b*gam+bet))*silu
                        for c in range(8):
                            g1 = gring.tile([128, UQ], BF16, tag="g1")
                            nc.vector.scalar_tensor_tensor(
                                g1[:], attnT[:, c, usl], gamsb[:, c:c + 1],
                                rstd_b[:], ALU.mult, ALU.mult)
                            g2 = gring.tile([128, UQ], BF16, tag="g2")
                            nc.vector.scalar_tensor_tensor(
                                g2[:], nmr_b[:], gamsb[:, c:c + 1], g1[:],
                                ALU.mult, ALU.add)
                            nc.vector.scalar_tensor_tensor(
                                attnT[:, c, usl], g2[:], betsb[:, c:c + 1],
                                silu16[:, c, usl],
                                ALU.add, ALU.mult)
                            yield

                def outproj_gen(u, oh):
                    wo = woring.tile([128, 8, UQ], BF16, tag="wo")
                    nc.sync.dma_start(wo[:], wo3[:, :, UQ * oh:UQ * (oh + 1)])
                    with tc.tile_pool(name=f"psP{u}{oh}", bufs=2,
                                      space="PSUM") as psPp:
                        for rw in range(4):
                            r0 = u * UQ + 128 * rw
                            xqn = oring.tile([128, UQ], F32, tag="xqn", bufs=1)
                            nc.sync.dma_start(
                                xqn[:], xq[r0:r0 + 128,
                                           UQ * oh:UQ * (oh + 1)])
                            psOut = psPp.tile([128, UQ], F32, tag="po")
                            for c in range(8):
                                st = attnT[:, c,
                                           u * UQ + 128 * rw:
                                           u * UQ + 128 * (rw + 1)]
                                nc.tensor.matmul(
                                    psOut[:], st, wo[:, c, :],
                                    start=(c == 0), stop=(c == 7))
                                if c == 3:
                                    yield
                            osb = oring.tile([128, UQ], F32, tag="osb")
                            nc.vector.tensor_add(osb[:], psOut[:], xqn[:])
                            nc.sync.dma_start(
                                out[r0:r0 + 128, UQ * oh:UQ * (oh + 1)],
                                osb[:])
                            yield

                def drive(chains, fillers, steps_per=1):
                    """Round-robin: per chain step, advance filler."""
                    fi = iter(fillers)
                    cur = None
                    for ch in chains:
                        for _ in ch:
                            for _ in range(steps_per):
                                if cur is None:
                                    cur = next(fi, None)
                                    if cur is None:
                                        break
                                if next(cur, StopIteration) is StopIteration:
                                    cur = None
                    # drain remaining fillers
                    while True:
                        if cur is None:
                            cur = next(fi, None)
                            if cur is None:
                                break
                        if next(cur, StopIteration) is StopIteration:
                            cur = None

                # 2a: u0 chains, V(h1) as filler (u0 needs only kb<8)
                msb = ph2.tile([128, 16, UQ], BF16)
                nc.sync.dma_start(msb[:], maskT)
                with tc.tile_pool(name="pvj", bufs=2, space="PSUM") as pvj:
                    drive([attn_chain_gen(0, hp) for hp in range(8)]
                          + [attn_chain_gen(1, 0)],
                          [v_group_gen(1, xh1, pvj)])
                # 2b: u1 chains, u0 tail as filler
                drive([attn_chain_gen(1, hp) for hp in range(1, 8)],
                      [stats_gen(0), ln_gen(0),
                       outproj_gen(0, 0), outproj_gen(0, 1)])
                for p in reversed(attn_scope):
                    p.__exit__(None, None, None)

                # 2c: u1 tail, fused ln+outproj (c-outer, 8 PSUM banks)
                u = 1
                usl = slice(u * UQ, (u + 1) * UQ)
                for _ in stats_gen(u):
                    pass
                with tc.tile_pool(name="psG1f", bufs=2,
                                  space="PSUM") as psGp:
                    psRb = psGp.tile([128, UQ], F32, tag="bc")
                    nc.tensor.matmul(psRb[:], orowF[0:1, :],
                                     statr[0:1, 3, :], start=True, stop=True)
                    rstd_b = bcast.tile([128, UQ], BF16, tag="rb")
                    nc.scalar.activation(rstd_b[:], psRb[:], AF.Copy)
                    psNb = psGp.tile([128, UQ], F32, tag="bc")
                    nc.tensor.matmul(psNb[:], orowF[0:1, :],
                                     statr[0:1, 2, :], start=True, stop=True)
                    nmr_b = bcast.tile([128, UQ], BF16, tag="nb")
                    nc.scalar.activation(nmr_b[:], psNb[:], AF.Copy)
                wo0 = woring.tile([128, 8, UQ], BF16, tag="wo")
                nc.sync.dma_start(wo0[:], wo3[:, :, 0:UQ])
                wo1 = woring.tile([128, 8, UQ], BF16, tag="wo1")
                nc.sync.dma_start(wo1[:], wo3[:, :, UQ:D])
                wos = (wo0, wo1)
                with tc.tile_pool(name="psPF", bufs=8,
                                  space="PSUM") as psPp:
                    psOut = [psPp.tile([128, UQ], F32, tag="po",
                                       name=f"pof{i}")
                             for i in range(8)]
                    for c in range(8):
                        g1 = gring.tile([128, UQ], BF16, tag="g1")
                        nc.vector.scalar_tensor_tensor(
                            g1[:], attnT[:, c, usl], gamsb[:, c:c + 1],
                            rstd_b[:], ALU.mult, ALU.mult)
                        g2 = gring.tile([128, UQ], BF16, tag="g2")
                        nc.vector.scalar_tensor_tensor(
                            g2[:], nmr_b[:], gamsb[:, c:c + 1], g1[:],
                            ALU.mult, ALU.add)
                        nc.vector.scalar_tensor_tensor(
                            attnT[:, c, usl], g2[:], betsb[:, c:c + 1],
                            silu16[:, c, usl], ALU.add, ALU.mult)
                        for rw in range(4):
                            st = attnT[:, c,
                                       u * UQ + 128 * rw:
                                       u * UQ + 128 * (rw + 1)]
                            for oh in range(2):
                                nc.tensor.matmul(
                                    psOut[rw * 2 + oh][:], st,
                                    wos[oh][:, c, :],
                                    start=(c == 0), stop=(c == 7))
                    for rw in range(4):
                        r0 = u * UQ + 128 * rw
                        xqn = oring.tile([128, D], F32, tag="xqf", bufs=1)
                        nc.sync.dma_start(xqn[:], xq[r0:r0 + 128, :])
                        for oh in range(2):
                            osb = oring.tile([128, UQ], F32, tag="osb")
                            nc.vector.tensor_add(
                                osb[:], psOut[rw * 2 + oh][:],
                                xqn[:, UQ * oh:UQ * (oh + 1)])
                            nc.sync.dma_start(
                                out[r0:r0 + 128, UQ * oh:UQ * (oh + 1)],
                                osb[:])
    nc.finalize()
    return nc


def _host_prep(x, attn_mask, W_proj, b_proj, ln_gamma, ln_beta, W_out, b_out):
    """Build the 8 per-core input maps."""
    import ml_dtypes
    bf16 = ml_dtypes.bfloat16

    x = np.asarray(x, dtype=np.float32)
    attn_mask = np.asarray(attn_mask)
    W_proj = np.ascontiguousarray(np.asarray(W_proj, dtype=np.float32))
    W_out = np.ascontiguousarray(np.asarray(W_out, dtype=np.float32))
    b_proj = np.asarray(b_proj, dtype=np.float32)
    b_out = np.asarray(b_out, dtype=np.float32)
    ln_gamma = np.asarray(ln_gamma, dtype=np.float32)
    ln_beta = np.asarray(ln_beta, dtype=np.float32)

    inv = 1.0 / (10000.0 ** (np.arange(0, HD, 2, dtype=np.float64) / HD))
    ang = np.outer(inv, np.arange(L, dtype=np.float64))       # [32, L]
    c64 = np.concatenate([np.cos(ang), np.cos(ang)], 0)
    s64 = np.concatenate([np.sin(ang), np.sin(ang)], 0)
    cosk = np.concatenate([c64, c64], 0).astype(np.float32)   # [128, L]
    sink = np.concatenate([s64, s64], 0).astype(np.float32)

    p2 = np.zeros((128, 128), dtype=np.float32)
    for base in (0, 64):
        for m in range(32):
            p2[base + m + 32, base + m] = -1.0
        for m in range(32, 64):
            p2[base + m - 32, base + m] = 1.0

    shared = dict(
        wproj=W_proj.astype(bf16), wout=W_out.astype(bf16),
        cosk=cosk.astype(bf16), sink=sink.astype(bf16),
        p2=p2.astype(bf16),
        bprojT=np.ascontiguousarray(b_proj.reshape(32, 128).T),
        gamT=np.ascontiguousarray(ln_gamma.reshape(8, 128).T),
        betT=np.ascontiguousarray(ln_beta.reshape(8, 128).T),
        bcol=b_proj.reshape(1, 4 * D).astype(bf16),
        ones128=np.ones((128, 1), np.float32).astype(bf16),
        onesrow=np.ones((1, UQ), np.float32).astype(bf16),
        onesrowF=np.ones((1, 128), np.float32),
        vbias=np.broadcast_to(b_proj[D:2 * D], (128, D)).astype(bf16),
    )

    in_maps = []
    for c in range(NCORES):
        b, g = divmod(c, 2)
        own = np.r_[0:512, 1536:2048] if g == 0 else np.r_[512:1536]
        xb = x[b]
        xqc = np.ascontiguousarray(xb[own])
        m = dict(shared)
        m["xkvT"] = np.ascontiguousarray(xb.T).astype(bf16)
        m["xqT"] = np.ascontiguousarray(xqc.T).astype(bf16)
        m["xq"] = xqc + b_out[None, :]
        m["cosq"] = np.ascontiguousarray(cosk[:, own]).astype(bf16)
        m["sinq"] = np.ascontiguousarray(sink[:, own]).astype(bf16)
        mk = np.zeros((16, 128, UQ), dtype=np.float32)
        am = attn_mask[b]
        for u in range(2):
            qg = own[u * UQ:(u + 1) * UQ]
            for kb in range(8 * u, 8 * u + 8):
                mk[kb] = am[qg][:, 128 * kb:128 * (kb + 1)].T
        m["maskT"] = np.ascontiguousarray(
            mk.transpose(1, 0, 2).astype(bf16))
        in_maps.append(m)
    return in_maps


def kernel(**inputs):
    if "nc" not in _CACHED:
        _CACHED["nc"] = _build()
    nc = _CACHED["nc"]
    in_maps = _host_prep(**inputs)
    res = run_bass_kernel_spmd(nc, in_maps, list(range(NCORES)))
    full = np.empty((B, L, D), dtype=np.float32)
    for c in range(NCORES):
        b, g = divmod(c, 2)
        o = res.results[c]["out"]
        if g == 0:
            full[b, 0:512] = o[0:512]
            full[b, 1536:2048] = o[512:1024]
        else:
            full[b, 512:1536] = o
    return full



# revision 47
# speedup vs baseline: 1.0887x; 1.0074x over previous
"""HSTU block kernel v4 for 8 TRN2 NeuronCores (nn_HSTU_66279935312625).

Sharding: 2 cores per batch (B=4). Core pair splits the 2048 rows
causally-balanced: core g=0 owns rows [0,512)+[1536,2048), g=1 owns
[512,1536). Every core recomputes K/V projections for all 2048 rows of
its batch (communication-free). The program is identical on all cores;
all per-core variation lives in host-prepared input data.

v4: generator-driven schedule. All matmul operands bf16 (PE weight
double-buffering), one [128,1024] sigmoid per 2-head kb step.
Emission order interleaves at kb-step granularity:
  phase1: K(h0) V(h0) K(h1) Q+U          (PE-bound, rope on Act/DVE/GpSimd)
  phase2a: attn(u0) chains with V(h1) groups as PE filler
  phase2b: attn(u1) chains with u0 stats/LN/out-proj as PE filler
  phase2c: u1 tail
Act sigmoid stream stays saturated; PE fills its slack with tail work.
"""
import numpy as np

import concourse.bacc as bacc
import concourse.tile as tile
from concourse import mybir
from concourse.bass_utils import run_bass_kernel_spmd

F32 = mybir.dt.float32
F32R = mybir.dt.float32r
BF16 = mybir.dt.bfloat16
AF = mybir.ActivationFunctionType
ALU = mybir.AluOpType

B, L, D, H, HD = 4, 2048, 1024, 16, 64
OWN = 1024          # rows owned per core
UQ = 512            # rows per q-unit
EXT = (8, 16)       # k-block extent per q-unit (128-row k blocks)
SCALE = HD ** -0.5
LN_EPS = 1e-8
NCORES = 8

_CACHED = {}


def _build():
    nc = bacc.Bacc("TRN2", target_bir_lowering=False, debug=False)

    xkvT = nc.dram_tensor("xkvT", [D, L], BF16, kind="ExternalInput").ap()
    xqT = nc.dram_tensor("xqT", [D, OWN], BF16, kind="ExternalInput").ap()
    xq = nc.dram_tensor("xq", [OWN, D], F32, kind="ExternalInput").ap()
    wproj = nc.dram_tensor("wproj", [D, 4 * D], BF16, kind="ExternalInput").ap()
    wout = nc.dram_tensor("wout", [D, D], BF16, kind="ExternalInput").ap()
    cosk = nc.dram_tensor("cosk", [128, L], BF16, kind="ExternalInput").ap()
    sink = nc.dram_tensor("sink", [128, L], BF16, kind="ExternalInput").ap()
    cosq = nc.dram_tensor("cosq", [128, OWN], BF16, kind="ExternalInput").ap()
    sinq = nc.dram_tensor("sinq", [128, OWN], BF16, kind="ExternalInput").ap()
    p2 = nc.dram_tensor("p2", [128, 128], BF16, kind="ExternalInput").ap()
    maskT = nc.dram_tensor("maskT", [128, 16, UQ], BF16,
                           kind="ExternalInput").ap()
    bprojT = nc.dram_tensor("bprojT", [128, 32], F32, kind="ExternalInput").ap()
    gamT = nc.dram_tensor("gamT", [128, 8], F32R, kind="ExternalInput").ap()
    betT = nc.dram_tensor("betT", [128, 8], F32R, kind="ExternalInput").ap()
    bcol = nc.dram_tensor("bcol", [1, 4 * D], BF16, kind="ExternalInput").ap()
    ones128 = nc.dram_tensor("ones128", [128, 1], BF16,
                             kind="ExternalInput").ap()
    onesrow = nc.dram_tensor("onesrow", [1, UQ], BF16,
                             kind="ExternalInput").ap()
    onesrowF = nc.dram_tensor("onesrowF", [1, 128], F32R,
                              kind="ExternalInput").ap()
    vbias = nc.dram_tensor("vbias", [128, D], BF16, kind="ExternalInput").ap()
    out = nc.dram_tensor("out", [OWN, D], F32, kind="ExternalOutput").ap()

    wp3 = wproj.rearrange("(t ki) n -> ki t n", ki=128)   # [128, 8, 4096]
    wo3 = wout.rearrange("(t ki) n -> ki t n", ki=128)    # [128, 8, 1024]
    xkv3 = xkvT.rearrange("(t ki) n -> ki t n", ki=128)   # [128, 8, 2048]
    xq3 = xqT.rearrange("(t ki) n -> ki t n", ki=128)     # [128, 8, 1024]

    with tile.TileContext(nc) as tc:
        with (
            tc.tile_pool(name="const", bufs=1) as cpool,
            tc.tile_pool(name="big", bufs=1) as big,
            tc.tile_pool(name="ph1x1", bufs=1) as ph1x1,
            tc.tile_pool(name="wring", bufs=2) as wring,
        ):
            ph1scope = [tc.tile_pool(name="rope", bufs=3),
                        tc.tile_pool(name="trig", bufs=1),
                        tc.tile_pool(name="ppj", bufs=6, space="PSUM"),
                        tc.tile_pool(name="prt", bufs=2, space="PSUM")]
            rope, trig, ppj, prt = [p.__enter__() for p in ph1scope]
            ph1pool = tc.tile_pool(name="ph1", bufs=1)
            ph1 = ph1pool.__enter__()
            # ---- first-needed DMAs first ----
            wk0 = wring.tile([128, 8, 128], BF16, tag="wk")
            nc.sync.dma_start(wk0[:], wp3[:, :, 3 * D:3 * D + 128])
            p2sb = cpool.tile([128, 128], BF16)
            nc.sync.dma_start(p2sb[:], p2)
            bprojsb = cpool.tile([128, 32], F32)
            nc.sync.dma_start(bprojsb[:], bprojT)
            cksb0 = trig.tile([128, 1024], BF16, tag="cksb")
            nc.sync.dma_start(cksb0[:], cosk[:, 0:1024])
            sksb0 = trig.tile([128, 1024], BF16, tag="sksb")
            nc.sync.dma_start(sksb0[:], sink[:, 0:1024])
            xh0 = ph1.tile([128, 8, 1024], BF16, tag="xh")
            for t in range(8):
                nc.sync.dma_start(xh0[:, t, :], xkv3[:, t, 0:1024])
            gamsb = cpool.tile([128, 8], F32R)
            nc.sync.dma_start(gamsb[:], gamT)
            betsb = cpool.tile([128, 8], F32R)
            nc.sync.dma_start(betsb[:], betT)
            o128 = cpool.tile([128, 1], BF16)
            nc.sync.dma_start(o128[:], ones128)
            orow = cpool.tile([1, UQ], BF16)
            nc.sync.dma_start(orow[:], onesrow)
            orowF = cpool.tile([1, 128], F32R)
            nc.sync.dma_start(orowF[:], onesrowF)
            vbsb = cpool.tile([128, D], BF16)
            nc.sync.dma_start(vbsb[:], vbias)
            epsb = cpool.tile([1, 1], F32)
            nc.vector.memset(epsb[:], LN_EPS)

            krot = big.tile([128, 8, L], BF16)      # K_rot^T
            v16 = big.tile([128, 16, D], BF16)      # V natural [row tiles]
            qrot = big.tile([128, 8, OWN], BF16)    # Q_rot^T
            silu16 = big.tile([128, 8, OWN], BF16)  # silu(U)^T

            def rope_chain(psP, bias, cos_sl, sin_sl, dst):
                # dst(bf16) = (psP+b)*cos + rotate_half(psP+b)*sin
                t16 = rope.tile([128, UQ], BF16, tag="t16")
                nc.scalar.activation(t16[:], psP[:], AF.Identity, bias=bias)
                psR = prt.tile([128, UQ], F32, tag="psR")
                nc.tensor.matmul(psR[:], p2sb[:], t16[:], start=True,
                                 stop=True)
                tcos = rope.tile([128, UQ], BF16, tag="tcos")
                nc.vector.scalar_tensor_tensor(
                    tcos[:], psP[:], bias, cos_sl, ALU.add, ALU.mult)
                tsin = rope.tile([128, UQ], BF16, tag="tsin")
                nc.vector.tensor_mul(tsin[:], psR[:], sin_sl)
                nc.vector.tensor_add(dst, tcos[:], tsin[:])

            def k_block(h, xh, cksb, sksb, first_wk=None):
                for ct in range(8):
                    c0 = 3 * D + 128 * ct
                    if first_wk is not None and ct == 0:
                        wk = first_wk
                    else:
                        wk = wring.tile([128, 8, 128], BF16, tag="wk")
                        nc.sync.dma_start(wk[:], wp3[:, :, c0:c0 + 128])
                    for r in range(2):
                        ps = ppj.tile([128, UQ], F32, tag="ps")
                        for t in range(8):
                            nc.tensor.matmul(
                                ps[:], wk[:, t, :],
                                xh[:, t, r * UQ:(r + 1) * UQ],
                                start=(t == 0), stop=(t == 7))
                        off = r * UQ
                        rope_chain(ps, bprojsb[:, 24 + ct:25 + ct],
                                   cksb[:, off:off + UQ],
                                   sksb[:, off:off + UQ],
                                   krot[:, ct, h * 1024 + off:
                                        h * 1024 + off + UQ])

            def v_group_gen(h, xh, vpool):
                for vh in range(2):
                    v0 = D + UQ * vh
                    wvh = wring.tile([128, 8, UQ], BF16, tag="wv", bufs=1)
                    nc.sync.dma_start(wvh[:], wp3[:, :, v0:v0 + UQ])
                    for rv in range(8):
                        grv = h * 8 + rv
                        pv = vpool.tile([128, UQ], F32, tag="ps")
                        for t in range(8):
                            nc.tensor.matmul(
                                pv[:], xh[:, t, 128 * rv:128 * (rv + 1)],
                                wvh[:, t, :], start=(t == 0), stop=(t == 7))
                            if t == 3:
                                yield
                        nc.vector.scalar_tensor_tensor(
                            v16[:, grv, UQ * vh:UQ * (vh + 1)], pv[:], 0.0,
                            vbsb[:, UQ * vh:UQ * (vh + 1)],
                            ALU.add, ALU.add)
                        yield

            # ---------- phase 1: K(h0), V(h0), K(h1), Q+U ----------
            xh1 = ph1x1.tile([128, 8, 1024], BF16, tag="xh1")
            nc.sync.dma_start(xh1[:], xkv3[:, :, 1024:2048])
            k_block(0, xh0, cksb0, sksb0, first_wk=wk0)
            for _ in v_group_gen(0, xh0, ppj):
                pass
            ph1pool.__exit__(None, None, None)
            cksb1 = trig.tile([128, 1024], BF16, tag="cksb")
            nc.sync.dma_start(cksb1[:], cosk[:, 1024:2048])
            sksb1 = trig.tile([128, 1024], BF16, tag="sksb")
            nc.sync.dma_start(sksb1[:], sink[:, 1024:2048])
            k_block(1, xh1, cksb1, sksb1)

            with (
                tc.tile_pool(name="ph1a", bufs=1) as ph1a,
                tc.tile_pool(name="wring2", bufs=2) as wring2,
            ):
                xqsb = ph1a.tile([128, 8, OWN], BF16)
                nc.sync.dma_start(xqsb[:], xq3)
                cqsb = ph1a.tile([128, OWN], BF16)
                nc.sync.dma_start(cqsb[:], cosq)
                sqsb = ph1a.tile([128, OWN], BF16)
                nc.sync.dma_start(sqsb[:], sinq)

                for ct in range(8):
                    wu = wring2.tile([128, 8, 128], BF16, tag="wu")
                    nc.sync.dma_start(wu[:], wp3[:, :, 128 * ct:128 * (ct + 1)])
                    q0 = 2 * D + 128 * ct
                    wq = wring2.tile([128, 8, 128], BF16, tag="wq")
                    nc.sync.dma_start(wq[:], wp3[:, :, q0:q0 + 128])
                    for r in range(2):
                        sl = slice(r * UQ, (r + 1) * UQ)
                        psu = ppj.tile([128, UQ], F32, tag="ps")
                        for t in range(8):
                            nc.tensor.matmul(psu[:], wu[:, t, :],
                                             xqsb[:, t, sl],
                                             start=(t == 0), stop=(t == 7))
                        nc.scalar.activation(silu16[:, ct, sl], psu[:],
                                             AF.Silu,
                                             bias=bprojsb[:, ct:ct + 1])
                        psq = ppj.tile([128, UQ], F32, tag="ps")
                        for t in range(8):
                            nc.tensor.matmul(psq[:], wq[:, t, :],
                                             xqsb[:, t, sl],
                                             start=(t == 0), stop=(t == 7))
                        rope_chain(psq, bprojsb[:, 16 + ct:17 + ct],
                                   cqsb[:, sl], sqsb[:, sl],
                                   qrot[:, ct, sl])

            for p in reversed(ph1scope):
                p.__exit__(None, None, None)

            # ---------- phase 2 ----------
            with (
                tc.tile_pool(name="ph2", bufs=1) as ph2,
                tc.tile_pool(name="aring", bufs=3) as aring,
                tc.tile_pool(name="sqring", bufs=1) as sqring,
                tc.tile_pool(name="bcast", bufs=1) as bcast,
                tc.tile_pool(name="gring", bufs=1) as gring,
                tc.tile_pool(name="oring", bufs=2) as oring,
                tc.tile_pool(name="woring", bufs=1) as woring,
            ):
                attn_scope = [tc.tile_pool(name="psS_", bufs=2, space="PSUM"),
                              tc.tile_pool(name="psO_", bufs=2, space="PSUM")]
                psSp, psOp = [p.__enter__() for p in attn_scope]
                attnT = ph2.tile([128, 8, L // 2], BF16)
                statr = ph2.tile([1, 4, UQ], F32R)
                tail_sb = {}

                def attn_chain_gen(u, hp):
                    psO = psOp.tile([128, UQ], F32, tag="psO")
                    qA = qrot[0:64, hp, u * UQ:(u + 1) * UQ]
                    qB = qrot[64:128, hp, u * UQ:(u + 1) * UQ]
                    kbs = range(EXT[u] - 1, -1, -1) if u == 1 else \
                        range(EXT[u])
                    for j, kb in enumerate(kbs):
                        psS = psSp.tile([128, 1024], F32, tag="psS")
                        nc.tensor.matmul(
                            psS[:, 0:UQ],
                            krot[0:64, hp, 128 * kb:128 * (kb + 1)],
                            qA, start=True, stop=True)
                        nc.tensor.matmul(
                            psS[:, UQ:1024],
                            krot[64:128, hp, 128 * kb:128 * (kb + 1)],
                            qB, start=True, stop=True)
                        aAB = aring.tile([128, 1024], BF16, tag="aAB")
                        nc.scalar.activation(aAB[:], psS[:], AF.Sigmoid,
                                             scale=SCALE)
                        mi = kb - 8 * u
                        if 0 <= mi < 8:
                            mj = 8 * u + mi
                            nc.vector.tensor_mul(aAB[:, 0:UQ], aAB[:, 0:UQ],
                                                 msb[:, mj, :])
                            nc.vector.tensor_mul(aAB[:, UQ:1024],
                                                 aAB[:, UQ:1024],
                                                 msb[:, mj, :])
                        last = j == EXT[u] - 1
                        nc.tensor.matmul(
                            psO[0:64, :],
                            v16[:, kb, 128 * hp:128 * hp + 64],
                            aAB[:, 0:UQ], start=(j == 0), stop=last,
                            tile_position=(0, 0))
                        nc.tensor.matmul(
                            psO[64:128, :],
                            v16[:, kb, 128 * hp + 64:128 * (hp + 1)],
                            aAB[:, UQ:1024], start=(j == 0), stop=last,
                            tile_position=(0, 64))
                        yield
                    nc.vector.tensor_copy(
                        attnT[:, hp, u * UQ:(u + 1) * UQ], psO[:])

                def stats_gen(u):
                    usl = slice(u * UQ, (u + 1) * UQ)
                    with tc.tile_pool(name=f"psT{u}", bufs=2,
                                      space="PSUM") as psTp:
                        psSum = psTp.tile([1, UQ], F32, tag="st")
                        psSq = psTp.tile([1, UQ], F32, tag="st")
                        for hp in range(8):
                            sq = sqring.tile([128, UQ], BF16, tag="sq")
                            nc.vector.tensor_mul(sq[:], attnT[:, hp, usl],
                                                 attnT[:, hp, usl])
                            nc.tensor.matmul(psSum[:], o128[:],
                                             attnT[:, hp, usl],
                                             start=(hp == 0), stop=(hp == 7))
                            nc.tensor.matmul(psSq[:], o128[:], sq[:],
                                             start=(hp == 0), stop=(hp == 7))
                            yield

                        mu = statr[0:1, 0, :]
                        nc.vector.tensor_scalar_mul(mu, psSum[:], 1.0 / D)
                        m2 = statr[0:1, 1, :]
                        nc.vector.tensor_scalar_mul(m2, psSq[:], 1.0 / D)
                        musq = statr[0:1, 2, :]
                        nc.vector.tensor_mul(musq, mu, mu)
                        varr = statr[0:1, 1, :]
                        nc.vector.tensor_sub(varr, m2, musq)
                        rstd = statr[0:1, 3, :]
                        nc.scalar.activation(rstd, varr, AF.Sqrt, bias=epsb[:])
                        with nc.allow_low_precision("f32r rstd for matmul"):
                            nc.vector.reciprocal(rstd, rstd)
                        nmr = statr[0:1, 2, :]
                        nc.vector.tensor_mul(nmr, mu, rstd)
                        nc.vector.tensor_scalar_mul(nmr, nmr, -1.0)
                        yield

                def ln_gen(u):
                    usl = slice(u * UQ, (u + 1) * UQ)
                    with tc.tile_pool(name=f"psG{u}", bufs=2,
                                      space="PSUM") as psGp:
                        psRb = psGp.tile([128, UQ], F32, tag="bc")
                        nc.tensor.matmul(psRb[:], orowF[0:1, :],
                                         statr[0:1, 3, :],
                                         start=True, stop=True)
                        rstd_b = bcast.tile([128, UQ], BF16, tag="rb")
                        nc.scalar.activation(rstd_b[:], psRb[:], AF.Copy)
                        psNb = psGp.tile([128, UQ], F32, tag="bc")
                        nc.tensor.matmul(psNb[:], orowF[0:1, :],
                                         statr[0:1, 2, :],
                                         start=True, stop=True)
                        nmr_b = bcast.tile([128, UQ], BF16, tag="nb")
                        nc.scalar.activation(nmr_b[:], psNb[:], AF.Copy)
                        yield

                        # gated = ((aT*gam)*rstd_b + (nmr_b*gam+bet))*silu
                        for c in range(8):
                            g1 = gring.tile([128, UQ], BF16, tag="g1")
                            nc.vector.scalar_tensor_tensor(
                                g1[:], attnT[:, c, usl], gamsb[:, c:c + 1],
                                rstd_b[:], ALU.mult, ALU.mult)
                            g2 = gring.tile([128, UQ], BF16, tag="g2")
                            nc.vector.scalar_tensor_tensor(
                                g2[:], nmr_b[:], gamsb[:, c:c + 1], g1[:],
                                ALU.mult, ALU.add)
                            nc.vector.scalar_tensor_tensor(
                                attnT[:, c, usl], g2[:], betsb[:, c:c + 1],
                                silu16[:, c, usl],
                                ALU.add, ALU.mult)
                            yield

                def outproj_gen(u, oh):
                    wo = woring.tile([128, 8, UQ], BF16, tag="wo")
                    nc.sync.dma_start(wo[:], wo3[:, :, UQ * oh:UQ * (oh + 1)])
                    with tc.tile_pool(name=f"psP{u}{oh}", bufs=2,
                                      space="PSUM") as psPp:
                        for rw in range(4):
                            r0 = u * UQ + 128 * rw
                            xqn = oring.tile([128, UQ], F32, tag="xqn", bufs=1)
                            nc.sync.dma_start(
                                xqn[:], xq[r0:r0 + 128,
                                           UQ * oh:UQ * (oh + 1)])
                            psOut = psPp.tile([128, UQ], F32, tag="po")
                            for c in range(8):
                                st = attnT[:, c,
                                           u * UQ + 128 * rw:
                                           u * UQ + 128 * (rw + 1)]
                                nc.tensor.matmul(
                                    psOut[:], st, wo[:, c, :],
                                    start=(c == 0), stop=(c == 7))
                                if c == 3:
                                    yield
                            osb = oring.tile([128, UQ], F32, tag="osb")
                            nc.vector.tensor_add(osb[:], psOut[:], xqn[:])
                            nc.sync.dma_start(
                                out[r0:r0 + 128, UQ * oh:UQ * (oh + 1)],
                                osb[:])
                            yield

                def drive(chains, fillers, steps_per=1):
                    """Round-robin: per chain step, advance filler."""
                    fi = iter(fillers)
                    cur = None
                    for ch in chains:
                        for _ in ch:
                            for _ in range(steps_per):
                                if cur is None:
                                    cur = next(fi, None)
                                    if cur is None:
                                        break
                                if next(cur, StopIteration) is StopIteration:
                                    cur = None
                    # drain remaining fillers
                    while True:
                        if cur is None:
                            cur = next(fi, None)
                            if cur is None:
                                break
                        if next(cur, StopIteration) is StopIteration:
                            cur = None

                # 2a: u0 chains, V(h1) as filler (u0 needs only kb<8)
                msb = ph2.tile([128, 16, UQ], BF16)
                nc.sync.dma_start(msb[:], maskT)
                with tc.tile_pool(name="pvj", bufs=2, space="PSUM") as pvj:
                    drive([attn_chain_gen(0, hp) for hp in range(8)]
                          + [attn_chain_gen(1, 0)],
                          [v_group_gen(1, xh1, pvj)])
                # 2b: u1 chains, u0 tail as filler
                drive([attn_chain_gen(1, hp) for hp in range(1, 8)],
                      [stats_gen(0), ln_gen(0),
                       outproj_gen(0, 0), outproj_gen(0, 1)])
                for p in reversed(attn_scope):
                    p.__exit__(None, None, None)

                # 2c: u1 tail, fused ln+outproj (c-outer, 8 PSUM banks)
                u = 1
                usl = slice(u * UQ, (u + 1) * UQ)
                for _ in stats_gen(u):
                    pass
                with tc.tile_pool(name="psG1f", bufs=2,
                                  space="PSUM") as psGp:
                    psRb = psGp.tile([128, UQ], F32, tag="bc")
                    nc.tensor.matmul(psRb[:], orowF[0:1, :],
                                     statr[0:1, 3, :], start=True, stop=True)
                    rstd_b = bcast.tile([128, UQ], BF16, tag="rb")
                    nc.scalar.activation(rstd_b[:], psRb[:], AF.Copy)
                    psNb = psGp.tile([128, UQ], F32, tag="bc")
                    nc.tensor.matmul(psNb[:], orowF[0:1, :],
                                     statr[0:1, 2, :], start=True, stop=True)
                    nmr_b = bcast.tile([128, UQ], BF16, tag="nb")
                    nc.scalar.activation(nmr_b[:], psNb[:], AF.Copy)
                wo0 = woring.tile([128, 8, UQ], BF16, tag="wo")
                nc.sync.dma_start(wo0[:], wo3[:, :, 0:UQ])
                wo1 = woring.tile([128, 8, UQ], BF16, tag="wo1")
                nc.sync.dma_start(wo1[:], wo3[:, :, UQ:D])
                wos = (wo0, wo1)
                with tc.tile_pool(name="psPF", bufs=8,
                                  space="PSUM") as psPp:
                    psOut = [psPp.tile([128, UQ], F32, tag="po",
                                       name=f"pof{i}")
                             for i in range(8)]
                    for c in range(8):
                        g1 = gring.tile([128, UQ], BF16, tag="g1")
                        nc.vector.scalar_tensor_tensor(
                            g1[:], attnT[:, c, usl], gamsb[:, c:c + 1],
                            rstd_b[:], ALU.mult, ALU.mult)
                        g2 = gring.tile([128, UQ], BF16, tag="g2")
                        nc.vector.scalar_tensor_tensor(
                            g2[:], nmr_b[:], gamsb[:, c:c + 1], g1[:],
                            ALU.mult, ALU.add)
                        nc.vector.scalar_tensor_tensor(
                            attnT[:, c, usl], g2[:], betsb[:, c:c + 1],
                            silu16[:, c, usl], ALU.add, ALU.mult)
                        for rw in range(4):
                            st = attnT[:, c,
                                       u * UQ + 128 * rw:
                                       u * UQ + 128 * (rw + 1)]
                            for oh in range(2):
                                nc.tensor.matmul(
                                    psOut[rw * 2 + oh][:], st,
                                    wos[oh][:, c, :],
                                    start=(c == 0), stop=(c == 7))
                    for rw in range(4):
                        r0 = u * UQ + 128 * rw
                        xqn = oring.tile([128, D], F32, tag="xqf", bufs=1)
                        nc.sync.dma_start(xqn[:], xq[r0:r0 + 128, :])
                        for oh in range(2):
                            osb = oring.tile([128, UQ], F32, tag="osb")
                            nc.vector.tensor_add(
                                osb[:], psOut[rw * 2 + oh][:],
                                xqn[:, UQ * oh:UQ * (oh + 1)])
                            nc.sync.dma_start(
                                out[r0:r0 + 128, UQ * oh:UQ * (oh + 1)],
                                osb[:])
    nc.finalize()
    return nc


def _host_prep(x, attn_mask, W_proj, b_proj, ln_gamma, ln_beta, W_out, b_out):
    """Build the 8 per-core input maps."""
    import ml_dtypes
    bf16 = ml_dtypes.bfloat16

    x = np.asarray(x, dtype=np.float32)
    attn_mask = np.asarray(attn_mask)
    W_proj = np.ascontiguousarray(np.asarray(W_proj, dtype=np.float32))
    W_out = np.ascontiguousarray(np.asarray(W_out, dtype=np.float32))
    b_proj = np.asarray(b_proj, dtype=np.float32)
    b_out = np.asarray(b_out, dtype=np.float32)
    ln_gamma = np.asarray(ln_gamma, dtype=np.float32)
    ln_beta = np.asarray(ln_beta, dtype=np.float32)

    inv = 1.0 / (10000.0 ** (np.arange(0, HD, 2, dtype=np.float64) / HD))
    ang = np.outer(inv, np.arange(L, dtype=np.float64))       # [32, L]
    c64 = np.concatenate([np.cos(ang), np.cos(ang)], 0)
    s64 = np.concatenate([np.sin(ang), np.sin(ang)], 0)
    cosk = np.concatenate([c64, c64], 0).astype(np.float32)   # [128, L]
    sink = np.concatenate([s64, s64], 0).astype(np.float32)

    p2 = np.zeros((128, 128), dtype=np.float32)
    for base in (0, 64):
        for m in range(32):
            p2[base + m + 32, base + m] = -1.0
        for m in range(32, 64):
            p2[base + m - 32, base + m] = 1.0

    shared = dict(
        wproj=W_proj.astype(bf16), wout=W_out.astype(bf16),
        cosk=cosk.astype(bf16), sink=sink.astype(bf16),
        p2=p2.astype(bf16),
        bprojT=np.ascontiguousarray(b_proj.reshape(32, 128).T),
        gamT=np.ascontiguousarray(ln_gamma.reshape(8, 128).T),
        betT=np.ascontiguousarray(ln_beta.reshape(8, 128).T),
        bcol=b_proj.reshape(1, 4 * D).astype(bf16),
        ones128=np.ones((128, 1), np.float32).astype(bf16),
        onesrow=np.ones((1, UQ), np.float32).astype(bf16),
        onesrowF=np.ones((1, 128), np.float32),
        vbias=np.broadcast_to(b_proj[D:2 * D], (128, D)).astype(bf16),
    )

    in_maps = []
    for c in range(NCORES):
        b, g = divmod(c, 2)
        own = np.r_[0:512, 1536:2048] if g == 0 else np.r_[512:1536]
        xb = x[b]
        xqc = np.ascontiguousarray(xb[own])
        m = dict(shared)
        m["xkvT"] = np.ascontiguousarray(xb.T).astype(bf16)
        m["xqT"] = np.ascontiguousarray(xqc.T).astype(bf16)
        m["xq"] = xqc + b_out[None, :]
        m["cosq"] = np.ascontiguousarray(cosk[:, own]).astype(bf16)
        m["sinq"] = np.ascontiguousarray(sink[:, own]).astype(bf16)
        mk = np.zeros((16, 128, UQ), dtype=np.float32)
        am = attn_mask[b]
        for u in range(2):
            qg = own[u * UQ:(u + 1) * UQ]
            for kb in range(8 * u, 8 * u + 8):
                mk[kb] = am[qg][:, 128 * kb:128 * (kb + 1)].T
        m["maskT"] = np.ascontiguousarray(
            mk.transpose(1, 0, 2).astype(bf16))
        in_maps.append(m)
    return in_maps


def kernel(**inputs):
    if "nc" not in _CACHED:
        _CACHED["nc"] = _build()
    nc = _CACHED["nc"]
    in_maps = _host_prep(**inputs)
    res = run_bass_kernel_spmd(nc, in_maps, list(range(NCORES)))
    full = np.empty((B, L, D), dtype=np.float32)
    for c in range(NCORES):
        b, g = divmod(c, 2)
        o = res.results[c]["out"]
        if g == 0:
            full[b, 0:512] = o[0:512]
            full[b, 1536:2048] = o[512:1024]
        else:
            full[b, 512:1536] = o
    return full

